# revision 10
# baseline (speedup 1.0000x reference)
"""Multi-level DWT (DB4) decomposition on 8 Trainium2 NeuronCores.

Strategy (v2: phase-packed 64-row groups, fp16)
-----------------------------------------------
Each level is a 4-tap stride-2 conv along the column axis:
    approx[t] = c0*xe[t] + c1*xo[t] + c2*xe[t+1] + c3*xo[t+1]
    detail[t] = c3*xe[t] - c2*xo[t] + c1*xe[t+1] - c0*xo[t+1]
with xe[t]=x[2t], xo[t]=x[2t+1]; wraparound pads at level 0, zero pads
deeper.  Rows shard across the 8 cores (512 rows/core), zero comms.

On-core layout: rows are processed in 8 groups of 64.  A group's xe
lives on partitions 0-63 and its xo on partitions 64-127 of one SBUF
region, so a SINGLE matmul pair computes approx AND detail for 64 rows:
    mm1: stationary W1 (c0/c1 into approx rows, c3/-c2 into detail rows)
         x moving tile[:, t : t+fd]
    mm2: stationary W2 (c2/c3, c1/-c0) x tile[:, t+1 : t+1+fd], PSUM-acc.
That streams 2 moving columns per output column -- half the tap-matmul
scheme's 4 -- so PE time is ~8*sum(L) cycles ~= 27 us/core.  Everything
on-chip is fp16 (1 cyc/row at any free size, ample precision: taps and
N(0,1) data, fp32 PSUM accumulate), and the host packs/casts IO to fp16
so DMA traffic halves to ~8.4 MB/core.

PSUM->SBUF drains are the bottleneck-to-balance: approx deinterleaves
(strided PSUM reads, partition-crossing copy for the odd phase) feed the
next level's tile, details cast into fp16 staging for DMA-out.  GPSIMD
has no PSUM port, so these split between Activation (xe + ~61% of
detail) and DVE (xo + rest) to run just under PE.  Weight reloads
amortize by issuing all W1 matmuls of a 4-bank PSUM supertile before
the W2 pass.  Detail staging packs group pairs (even g -> partitions
0-63, odd g -> 64-127) so DMA-out rows land contiguously.
"""
import sys

if "/opt/trn_rl_repo" not in sys.path:
    sys.path.insert(0, "/opt/trn_rl_repo")

import numpy as np

import concourse.bacc as bacc
import concourse.mybir as mybir
from concourse import tile
from concourse.bass_utils import run_bass_kernel_spmd

DB4 = [0.4829629131445341, 0.8365163037378079, 0.2241438680420134,
       -0.1294095225512604]

B, N = 4096, 4096
NCORES = 8
RPC = B // NCORES        # rows per core = 512
P = 128                  # partitions
G = 8                    # row groups per core (64 rows each)
NLEV = 11                # int(log2(N)) - 1
W0 = N // 2 + 1          # level-0 per-group region width (xe|xo + pad)

F32 = mybir.dt.float32
F16 = mybir.dt.float16

ST = 1024                # PSUM supertile columns (fp32) = ST/512 banks
PSB = 16384 // (ST * 4)  # PSUM pool buffers

_nc_cache = {}


def build_program(loop_iters=None, variant="full"):
    """Build + compile the per-core Bass program (identical on all cores).

    loop_iters: if given, wrap the body in tc.For_i for wall-clock timing
    amplification (used by test.py, not by the grading path).
    variant: "full" | "mm" (matmuls only, timing diagnostics).
    """
    key = (loop_iters, variant)
    if key in _nc_cache:
        return _nc_cache[key]
    mm_only = variant == "mm"

    nc = bacc.Bacc("TRN2", target_bir_lowering=False, debug=False)
    x_d = nc.dram_tensor("x", [P, G * W0], F16, kind="ExternalInput").ap()
    w_d = nc.dram_tensor("w", [P, 256], F16, kind="ExternalInput").ap()
    y_d = nc.dram_tensor("y", [RPC, N], F16, kind="ExternalOutput").ap()

    with tile.TileContext(nc) as tc:
        with tc.tile_pool(name="sb", bufs=1) as sb, \
             tc.tile_pool(name="ps", bufs=PSB, space="PSUM") as ps:
            a_t = sb.tile([P, G * W0], F16, name="a_t")        # lev 0,2,4..
            b_t = sb.tile([P, G * (N // 4 + 1)], F16, name="b_t")  # 1,3,5..
            d0_t = sb.tile([P, 4 * (N // 2)], F16, name="d0_t")    # lev0 det
            d1_t = sb.tile([P, 4 * (N // 4)], F16, name="d1_t")    # lev1 det
            t2_t = sb.tile([P, 4 * (N // 4)], F16, name="t2_t")    # cols<1024
            w_t = sb.tile([P, 256], F16, name="w_t")
            z_t = sb.tile([P, 2], F16, name="z_t")

            yv = y_d.rearrange("(gg q) c -> q gg c", q=P)      # [128,4,4096]

            def body(_iv=None):
                nc.vector.memset(z_t[:], 0.0)
                nc.sync.dma_start(w_t[:], w_d)
                # group-0 slab in 4 pieces so PE starts after ~130 KB
                pieces = [(0, 514), (514, 1026), (1026, 1538), (1538, W0)]
                pieces += [(g * W0, (g + 1) * W0) for g in range(1, G)]
                for lo, hi in pieces:
                    nc.sync.dma_start(a_t[:, lo:hi], x_d[:, lo:hi])

                for lev in range(NLEV):
                    Li = N >> lev
                    Fi = Li >> 1             # outputs per parity per row
                    Fn = Fi >> 1
                    # src region width: host pad at lev0, truncation pad only
                    # where this level is group-batched (Fi < 512)
                    Wi = Fi + 1 if (lev == 0 or Fi < 512) else Fi
                    Wn = Fn + 1 if Fn < 512 else Fn
                    src_t = a_t if (lev % 2 == 0 or mm_only) else b_t
                    dst_t = b_t if lev % 2 == 0 else a_t
                    last = lev == NLEV - 1
                    if lev == 0:
                        det_t, det_w, det_b = d0_t, N // 2, 0
                    elif lev == 1:
                        det_t, det_w, det_b = d1_t, N // 4, 0
                    else:
                        det_t, det_w, det_b = t2_t, N // 4, Fi
                    # shorten the last W2 chunk when reading pad-less regions
                    short2 = lev >= 1 and Fi >= 512
                    ng = min(G, max(1, ST // Fi))     # groups per supertile
                    tw = min(Fi, ST)                  # cols per supertile
                    gb = max(1, min(ng, 512 // Fi))   # groups per matmul
                    sv = src_t[:, 0:G * Wi].rearrange("p (g w) -> p g w", g=G)
                    if not last:
                        dv = dst_t[:, 0:G * Wn].rearrange(
                            "p (g w) -> p g w", g=G)
                        if not mm_only and Wn == Fn + 1:
                            # zero truncation pads for a batched next level
                            nc.scalar.copy(
                                dv[:, :, Fn:Fn + 1],
                                z_t[:, 0:1].unsqueeze(1).to_broadcast(
                                    [P, G, 1]))
                    ev = det_t[:, 0:4 * det_w].rearrange(
                        "p (gg c) -> p gg c", gg=4)
                    # DVE takes xe + xo (strided psum reads are cheap there)
                    # + the detail tail; Act takes ~78% of detail
                    asp = min(tw, (int(0.78 * tw) + 1) & ~1)

                    if Fi >= ST:
                        sts = [(g, 1, t0) for g in range(G)
                               for t0 in range(0, Fi, ST)]
                    else:
                        sts = [(ga, ng, 0) for ga in range(0, G, ng)]
                    for ga, ngs, t0 in sts:
                        pt = ps.tile([P, ngs * tw], F32, name="pst", tag="ps")
                        # all W1 matmuls, then all W2 (amortize LD_WEIGHTS)
                        for sh, wsl in ((0, w_t[:, 0:128]),
                                        (1, w_t[:, 128:256])):
                            if Fi >= 512:
                                for gi in range(ngs):
                                    for c0 in range(t0, t0 + tw, 512):
                                        o = gi * tw + c0 - t0
                                        fd = 512
                                        if (sh == 1 and short2
                                                and c0 == Fi - 512):
                                            fd = 511
                                        nc.tensor.matmul(
                                            pt[:, o:o + fd], wsl,
                                            sv[:, ga + gi:ga + gi + 1,
                                               sh + c0:sh + c0 + fd],
                                            start=(sh == 0), stop=(sh == 1),
                                            skip_group_check=short2)
                            else:
                                for gm in range(0, ngs, gb):
                                    nc.tensor.matmul(
                                        pt[:, gm * Fi:(gm + gb) * Fi], wsl,
                                        sv[:, ga + gm:ga + gm + gb,
                                           sh:sh + Fi],
                                        start=(sh == 0), stop=(sh == 1))
                        if mm_only:
                            continue
                        pv = pt[:].rearrange("p (g f) -> p g f", g=ngs)
                        if last:
                            # final approx (2 cols, natural order) -> cols 0:2
                            nc.scalar.copy(ev[0:64, :, 0:2],
                                           pv[0:64, 0::2, :])
                            nc.vector.tensor_copy(ev[64:128, :, 0:2],
                                                  pv[0:64, 1::2, :])
                        else:
                            # approx, phase-split for the next level
                            h0 = t0 // 2
                            nc.vector.tensor_copy(
                                dv[0:64, ga:ga + ngs, h0:h0 + tw // 2],
                                pv[0:64, :, 0:tw:2])
                            nc.vector.tensor_copy(
                                dv[64:128, ga:ga + ngs, h0:h0 + tw // 2],
                                pv[0:64, :, 1:tw:2])
                        # details -> staging (group pairs pack 128 partitions)
                        if ngs == 1:
                            p2, gg = ga % 2, ga // 2
                            db = det_b + t0
                            nc.scalar.copy(
                                ev[p2 * 64:p2 * 64 + 64, gg:gg + 1,
                                   db:db + asp],
                                pv[64:128, 0:1, 0:asp])
                            nc.vector.tensor_copy(
                                ev[p2 * 64:p2 * 64 + 64, gg:gg + 1,
                                   db + asp:db + tw],
                                pv[64:128, 0:1, asp:tw])
                        else:
                            for p2 in (0, 1):
                                gg = (ga + p2) // 2
                                n2 = ngs // 2
                                dsl = ev[p2 * 64:p2 * 64 + 64, gg:gg + n2]
                                ssl = pv[64:128, p2::2]
                                nc.scalar.copy(
                                    dsl[:, :, det_b:det_b + asp],
                                    ssl[:, :, 0:asp])
                                if asp < tw:
                                    nc.vector.tensor_copy(
                                        dsl[:, :, det_b + asp:det_b + tw],
                                        ssl[:, :, asp:tw])

                    if mm_only:
                        continue
                    # stream details out as soon as a level completes
                    if lev == 0:
                        e0 = d0_t[:].rearrange("p (gg c) -> p gg c", gg=4)
                        nc.sync.dma_start(yv[:, 0:2, N // 2:N], e0[:, 0:2])
                        nc.sync.dma_start(yv[:, 2:4, N // 2:N], e0[:, 2:4])
                    elif lev == 1:
                        e1 = d1_t[:].rearrange("p (gg c) -> p gg c", gg=4)
                        nc.sync.dma_start(yv[:, 0:2, N // 4:N // 2], e1[:, 0:2])
                        nc.sync.dma_start(yv[:, 2:4, N // 4:N // 2], e1[:, 2:4])
                    elif Fi >= 64:
                        nc.sync.dma_start(yv[:, :, Fi:2 * Fi],
                                          ev[:, :, Fi:2 * Fi])
                if not mm_only:
                    # remnant: levels with Fi < 64 plus the final approx
                    tv = t2_t[:].rearrange("p (gg c) -> p gg c", gg=4)
                    nc.sync.dma_start(yv[:, :, 0:64], tv[:, :, 0:64])

            if loop_iters is None:
                body()
            else:
                with tc.For_i(0, loop_iters, 1,
                              hint_engines=(mybir.EngineType.PE,)) as iv:
                    body(iv)

    nc.compile()
    _nc_cache[key] = nc
    return nc


def _taps(W=None):
    if W is None:
        return list(DB4)
    W = np.asarray(W)
    return [float(W[i, 0]) for i in range(4)]


def _wmats(c):
    """[128, 256] fp16: [W1 | W2] stationaries (see module docstring)."""
    w = np.zeros((P, 256), dtype=np.float32)
    r = np.arange(64)
    w[r, r] = c[0]
    w[64 + r, r] = c[1]
    w[r, 64 + r] = c[3]
    w[64 + r, 64 + r] = -c[2]
    w[r, 128 + r] = c[2]
    w[64 + r, 128 + r] = c[3]
    w[r, 192 + r] = c[1]
    w[64 + r, 192 + r] = -c[0]
    return w.astype(np.float16)


def _pack_input(x):
    """[RPC, N] fp32 -> [128, G*W0] fp16 phase-packed groups with wrap pads."""
    xr = x.reshape(G, 64, N)
    out = np.empty((P, G, W0), dtype=np.float16)
    out[0:64, :, 0:N // 2] = xr[:, :, 0::2].transpose(1, 0, 2)
    out[64:128, :, 0:N // 2] = xr[:, :, 1::2].transpose(1, 0, 2)
    out[0:64, :, N // 2] = xr[:, :, 0].T
    out[64:128, :, N // 2] = xr[:, :, 1].T
    return out.reshape(P, G * W0)


def make_in_maps(input, W=None):
    x = np.ascontiguousarray(np.asarray(input), dtype=np.float32)
    assert x.shape == (B, N), x.shape
    w_np = _wmats(_taps(W))
    return [{"x": _pack_input(x[c * RPC:(c + 1) * RPC]), "w": w_np}
            for c in range(NCORES)]


def kernel(input, W=None, **_unused):
    in_maps = make_in_maps(input, W)
    nc = build_program()
    res = run_bass_kernel_spmd(nc, in_maps, core_ids=list(range(NCORES)))
    out = np.concatenate([res.results[c]["y"].astype(np.float32)
                          for c in range(NCORES)], axis=0)
    return np.ascontiguousarray(out, dtype=np.float32)


# revision 21
# speedup vs baseline: 1.2042x; 1.2042x over previous
"""Multi-level DWT (DB4) decomposition on 8 Trainium2 NeuronCores.

Strategy (v2: phase-packed 64-row groups, fp16)
-----------------------------------------------
Each level is a 4-tap stride-2 conv along the column axis:
    approx[t] = c0*xe[t] + c1*xo[t] + c2*xe[t+1] + c3*xo[t+1]
    detail[t] = c3*xe[t] - c2*xo[t] + c1*xe[t+1] - c0*xo[t+1]
with xe[t]=x[2t], xo[t]=x[2t+1]; wraparound pads at level 0, zero pads
deeper.  Rows shard across the 8 cores (512 rows/core), zero comms.

On-core layout: rows are processed in 8 groups of 64.  A group's xe
lives on partitions 0-63 and its xo on partitions 64-127 of one SBUF
region, so a SINGLE matmul pair computes approx AND detail for 64 rows:
    mm1: stationary W1 (c0/c1 into approx rows, c3/-c2 into detail rows)
         x moving tile[:, t : t+fd]
    mm2: stationary W2 (c2/c3, c1/-c0) x tile[:, t+1 : t+1+fd], PSUM-acc.
That streams 2 moving columns per output column -- half the tap-matmul
scheme's 4 -- so PE time is ~8*sum(L) cycles ~= 27 us/core.  Everything
on-chip is fp16 (1 cyc/row at any free size, ample precision: taps and
N(0,1) data, fp32 PSUM accumulate), and the host packs/casts IO to fp16
so DMA traffic halves to ~8.4 MB/core.

PSUM->SBUF drains are the bottleneck-to-balance: approx deinterleaves
(strided PSUM reads, partition-crossing copy for the odd phase) feed the
next level's tile, details cast into fp16 staging for DMA-out.  GPSIMD
has no PSUM port, so these split between Activation (xe + ~61% of
detail) and DVE (xo + rest) to run just under PE.  Weight reloads
amortize by issuing all W1 matmuls of a 4-bank PSUM supertile before
the W2 pass.  Detail staging packs group pairs (even g -> partitions
0-63, odd g -> 64-127) so DMA-out rows land contiguously.
"""
import sys

if "/opt/trn_rl_repo" not in sys.path:
    sys.path.insert(0, "/opt/trn_rl_repo")

import numpy as np

import concourse.bacc as bacc
import concourse.mybir as mybir
from concourse import tile
from concourse.bass_utils import run_bass_kernel_spmd

DB4 = [0.4829629131445341, 0.8365163037378079, 0.2241438680420134,
       -0.1294095225512604]

B, N = 4096, 4096
NCORES = 8
RPC = B // NCORES        # rows per core = 512
P = 128                  # partitions
G = 8                    # row groups per core (64 rows each)
NLEV = 11                # int(log2(N)) - 1
W0 = N // 2 + 1          # level-0 per-group region width (xe|xo + pad)

F32 = mybir.dt.float32
F16 = mybir.dt.float16

ST = 1024                # PSUM supertile columns (fp32) = ST/512 banks
PSB = 16384 // (ST * 4)  # PSUM pool buffers

_nc_cache = {}


def build_program(loop_iters=None, variant="full"):
    """Build + compile the per-core Bass program (identical on all cores).

    loop_iters: if given, wrap the body in tc.For_i for wall-clock timing
    amplification (used by test.py, not by the grading path).
    variant: "full" | "mm" (matmuls only, timing diagnostics).
    """
    key = (loop_iters, variant)
    if key in _nc_cache:
        return _nc_cache[key]
    mm_only = variant == "mm"

    nc = bacc.Bacc("TRN2", target_bir_lowering=False, debug=False)
    x_d = nc.dram_tensor("x", [P, G * W0], F16, kind="ExternalInput").ap()
    w_d = nc.dram_tensor("w", [P, 256], F16, kind="ExternalInput").ap()
    y_d = nc.dram_tensor("y", [RPC, N], F16, kind="ExternalOutput").ap()

    with tile.TileContext(nc) as tc:
        with tc.tile_pool(name="sb", bufs=1) as sb, \
             tc.tile_pool(name="ps", bufs=PSB, space="PSUM") as ps:
            x_t = sb.tile([P, G * W0], F16, name="x_t")        # lev-0 input
            a_t = sb.tile([P, G * (N // 8)], F16, name="a_t")  # lev 1,3,5..
            b_t = sb.tile([P, G * (N // 4)], F16, name="b_t")  # lev 0,2,4..
            d0_t = sb.tile([P, 4 * (N // 2)], F16, name="d0_t")    # lev0 det
            d1_t = sb.tile([P, 4 * (N // 4)], F16, name="d1_t")    # lev1 det
            t2_t = sb.tile([P, 4 * (N // 4)], F16, name="t2_t")    # cols<1024
            w_t = sb.tile([P, 256], F16, name="w_t")
            z_t = sb.tile([P, 2], F16, name="z_t")

            yv = y_d.rearrange("(gg q) c -> q gg c", q=P)      # [128,4,4096]

            def body(_iv=None):
                nc.vector.memset(z_t[:], 0.0)
                # input DMAs ride the idle Pool engine's SWDGE queue: the SP
                # queue would head-of-line block them behind the previous
                # iteration's output DMAs, serializing the input load
                nc.gpsimd.dma_start(w_t[:], w_d)
                # group-0 slab in 4 pieces so PE starts after ~130 KB
                pieces = [(0, 514), (514, 1026), (1026, 1538), (1538, W0)]
                pieces += [(g * W0, (g + 1) * W0) for g in range(1, G)]
                for lo, hi in pieces:
                    nc.gpsimd.dma_start(x_t[:, lo:hi], x_d[:, lo:hi])

                for lev in range(NLEV):
                    Li = N >> lev
                    Fi = Li >> 1             # outputs per parity per row
                    Fn = Fi >> 1
                    # src region width: host pad at lev0, truncation pad only
                    # where this level is group-batched (Fi < 512)
                    Wi = Fi + 1 if (lev == 0 or Fi < 512) else Fi
                    Wn = Fn + 1 if Fn < 512 else Fn
                    if lev == 0 or mm_only:
                        src_t = x_t
                    else:
                        src_t = b_t if lev % 2 == 1 else a_t
                    dst_t = b_t if lev % 2 == 0 else a_t
                    last = lev == NLEV - 1
                    if lev == 0:
                        det_t, det_w, det_b = d0_t, N // 2, 0
                    elif lev == 1:
                        det_t, det_w, det_b = d1_t, N // 4, 0
                    else:
                        det_t, det_w, det_b = t2_t, N // 4, Fi
                    # shorten the last W2 chunk when reading pad-less regions
                    short2 = lev >= 1 and Fi >= 512
                    ng = min(G, max(1, ST // Fi))     # groups per supertile
                    tw = min(Fi, ST)                  # cols per supertile
                    gb = max(1, min(ng, 512 // Fi))   # groups per matmul
                    sv = src_t[:, 0:G * Wi].rearrange("p (g w) -> p g w", g=G)
                    if not last:
                        dv = dst_t[:, 0:G * Wn].rearrange(
                            "p (g w) -> p g w", g=G)
                        if not mm_only and Wn == Fn + 1:
                            # zero truncation pads for a batched next level
                            # (Pool: its stall on the WAR dep must not block
                            # the Act det-copy stream)
                            nc.gpsimd.memset(dv[:, :, Fn:Fn + 1], 0.0)
                    ev = det_t[:, 0:4 * det_w].rearrange(
                        "p (gg c) -> p gg c", gg=4)
                    # copies cost ~1 cyc per SOURCE elem on either engine, so
                    # the strided deinterleaves split across engines where the
                    # Act AP limits allow (2-dim only): DVE xe always; Act xo
                    # at the big single-group levels, DVE xo deeper; detail
                    # balances the rest
                    asp = min(tw, (int(0.70 * tw) + 1) & ~1)

                    if Fi >= ST:
                        sts = [(g, 1, t0) for g in range(G)
                               for t0 in range(0, Fi, ST)]
                    else:
                        sts = [(ga, ng, 0) for ga in range(0, G, ng)]
                    for ga, ngs, t0 in sts:
                        pt = ps.tile([P, ngs * tw], F32, name="pst", tag="ps")
                        # all W1 matmuls, then all W2 (amortize LD_WEIGHTS)
                        for sh, wsl in ((0, w_t[:, 0:128]),
                                        (1, w_t[:, 128:256])):
                            if Fi >= 512:
                                for gi in range(ngs):
                                    for c0 in range(t0, t0 + tw, 512):
                                        o = gi * tw + c0 - t0
                                        fd = 512
                                        if (sh == 1 and short2
                                                and c0 == Fi - 512):
                                            fd = 511
                                        nc.tensor.matmul(
                                            pt[:, o:o + fd], wsl,
                                            sv[:, ga + gi:ga + gi + 1,
                                               sh + c0:sh + c0 + fd],
                                            start=(sh == 0), stop=(sh == 1),
                                            skip_group_check=short2)
                            else:
                                for gm in range(0, ngs, gb):
                                    nc.tensor.matmul(
                                        pt[:, gm * Fi:(gm + gb) * Fi], wsl,
                                        sv[:, ga + gm:ga + gm + gb,
                                           sh:sh + Fi],
                                        start=(sh == 0), stop=(sh == 1))
                        if mm_only:
                            continue
                        pv = pt[:].rearrange("p (g f) -> p g f", g=ngs)
                        if last:
                            # final approx (2 cols, natural order) -> cols 0:2
                            nc.scalar.copy(ev[0:64, :, 0:2],
                                           pv[0:64, 0::2, :])
                            nc.vector.tensor_copy(ev[64:128, :, 0:2],
                                                  pv[0:64, 1::2, :])
                        else:
                            # approx, phase-split for the next level
                            h0 = t0 // 2
                            nc.vector.tensor_copy(
                                dv[0:64, ga:ga + ngs, h0:h0 + tw // 2],
                                pv[0:64, :, 0:tw:2])
                            if ngs == 1:
                                # flat 2-dim AP so it can ride Activation
                                fb = ga * Wn + h0
                                nc.scalar.copy(
                                    dst_t[64:128, fb:fb + tw // 2],
                                    pt[0:64, 1:tw:2])
                            else:
                                nc.vector.tensor_copy(
                                    dv[64:128, ga:ga + ngs, h0:h0 + tw // 2],
                                    pv[0:64, :, 1:tw:2])
                        # details -> staging (group pairs pack 128 partitions)
                        if ngs == 1:
                            p2, gg = ga % 2, ga // 2
                            db = det_b + t0
                            nc.scalar.copy(
                                ev[p2 * 64:p2 * 64 + 64, gg:gg + 1,
                                   db:db + asp],
                                pv[64:128, 0:1, 0:asp])
                            if asp < tw:
                                nc.vector.tensor_copy(
                                    ev[p2 * 64:p2 * 64 + 64, gg:gg + 1,
                                       db + asp:db + tw],
                                    pv[64:128, 0:1, asp:tw])
                        else:
                            for p2 in (0, 1):
                                gg = (ga + p2) // 2
                                n2 = ngs // 2
                                dsl = ev[p2 * 64:p2 * 64 + 64, gg:gg + n2]
                                ssl = pv[64:128, p2::2]
                                nc.scalar.copy(
                                    dsl[:, :, det_b:det_b + asp],
                                    ssl[:, :, 0:asp])
                                if asp < tw:
                                    nc.vector.tensor_copy(
                                        dsl[:, :, det_b + asp:det_b + tw],
                                        ssl[:, :, asp:tw])

                    if mm_only:
                        continue
                    # stream details out as soon as a level completes
                    if lev == 0:
                        e0 = d0_t[:].rearrange("p (gg c) -> p gg c", gg=4)
                        nc.sync.dma_start(yv[:, 0:2, N // 2:N], e0[:, 0:2])
                        nc.sync.dma_start(yv[:, 2:4, N // 2:N], e0[:, 2:4])
                    elif lev == 1:
                        e1 = d1_t[:].rearrange("p (gg c) -> p gg c", gg=4)
                        nc.sync.dma_start(yv[:, 0:2, N // 4:N // 2], e1[:, 0:2])
                        nc.sync.dma_start(yv[:, 2:4, N // 4:N // 2], e1[:, 2:4])
                    elif Fi >= 64:
                        nc.sync.dma_start(yv[:, :, Fi:2 * Fi],
                                          ev[:, :, Fi:2 * Fi])
                if not mm_only:
                    # remnant: levels with Fi < 64 plus the final approx
                    tv = t2_t[:].rearrange("p (gg c) -> p gg c", gg=4)
                    nc.sync.dma_start(yv[:, :, 0:64], tv[:, :, 0:64])

            if loop_iters is None:
                body()
            else:
                with tc.For_i(0, loop_iters, 1,
                              hint_engines=(mybir.EngineType.PE,)) as iv:
                    body(iv)

    nc.compile()
    _nc_cache[key] = nc
    return nc


def _taps(W=None):
    if W is None:
        return list(DB4)
    W = np.asarray(W)
    return [float(W[i, 0]) for i in range(4)]


def _wmats(c):
    """[128, 256] fp16: [W1 | W2] stationaries (see module docstring)."""
    w = np.zeros((P, 256), dtype=np.float32)
    r = np.arange(64)
    w[r, r] = c[0]
    w[64 + r, r] = c[1]
    w[r, 64 + r] = c[3]
    w[64 + r, 64 + r] = -c[2]
    w[r, 128 + r] = c[2]
    w[64 + r, 128 + r] = c[3]
    w[r, 192 + r] = c[1]
    w[64 + r, 192 + r] = -c[0]
    return w.astype(np.float16)


def _pack_input(x):
    """[RPC, N] fp32 -> [128, G*W0] fp16 phase-packed groups with wrap pads."""
    xr = x.reshape(G, 64, N)
    out = np.empty((P, G, W0), dtype=np.float16)
    out[0:64, :, 0:N // 2] = xr[:, :, 0::2].transpose(1, 0, 2)
    out[64:128, :, 0:N // 2] = xr[:, :, 1::2].transpose(1, 0, 2)
    out[0:64, :, N // 2] = xr[:, :, 0].T
    out[64:128, :, N // 2] = xr[:, :, 1].T
    return out.reshape(P, G * W0)


def make_in_maps(input, W=None):
    x = np.ascontiguousarray(np.asarray(input), dtype=np.float32)
    assert x.shape == (B, N), x.shape
    w_np = _wmats(_taps(W))
    return [{"x": _pack_input(x[c * RPC:(c + 1) * RPC]), "w": w_np}
            for c in range(NCORES)]


def kernel(input, W=None, **_unused):
    in_maps = make_in_maps(input, W)
    nc = build_program()
    res = run_bass_kernel_spmd(nc, in_maps, core_ids=list(range(NCORES)))
    out = np.concatenate([res.results[c]["y"].astype(np.float32)
                          for c in range(NCORES)], axis=0)
    return np.ascontiguousarray(out, dtype=np.float32)
